# revision 25
# baseline (speedup 1.0000x reference)
"""Trainium2 Bass kernel for AngularMinPooling.

out[v, r] = inputs[v, r, argmin_j ||inputs[v, j, :]||_2]
Input (500000, 8, 64) f32 -> Output (500000, 8) f32.
Vertices are sharded across 8 NeuronCores; no cross-core communication.

The problem is memory-bound: 128 MB of input per core against the
per-core HBM share (~358 GB/s nominal, ~373 GB/s delivered here), so
the whole design is about keeping the 16 SDMA queues saturated while
every compute engine runs with slack. Per 128x8-vertex tile: ACT
squares the features into a scratch tile (f32 end-to-end: fp16/bf16
squares flip ~0.02-0.15% of argmins and push rel-err to 1.7e-2..5e-2,
too close to the 2e-2 gate) and copies the first R feature columns to a
small f32 gather tile. The copy exists so the big input buffer frees
after two ACT-stages -- when the gather instead read the input tile
directly, the buffer lived through the whole 6-stage chain, the DMA
runway shrank to ~2 tiles, and the sync sequencer spent ~68 us
head-of-line blocked on buffer-free semaphores (433 us vs 343). DVE
does only single-port work -- the big segmented f32 sum-reduce to
squared norms (4.0 us/tile, the dominant DVE cost; tensor_reduce has
only a 1x uop so 16-bit inputs would not speed it up), the min-reduce,
and the small is_le one-hot -- while the one-hot gather (mult plus a
3-level add tree, measured 2.7 cyc/elem) runs on GpSimd. GpSimd shares
only DVE's *second* SBUF port and tensor_reduce is a 1-port
instruction, so the two engines stream concurrently without contention
(is_le is the lone 2-port DVE op; it must stay on DVE -- comparison
ALU ops on the Pool engine fail to compile). Input DMAs are issued
PREFETCH=2 blocks ahead of their compute so the triggers on the scalar
ring never queue behind ACT's square/copy dispatch (this prefetch took
410 us to 343 us). That leaves DVE ~86%, ACT ~79%, GpSimd ~44% busy
with DMA queues 91% occupied: DMA is the bottleneck and the pipeline
has no stall limit-cycle. Output is staged in f32 SBUF and written out
in chunks on the sync ring one block after each region completes,
partition-major (the host undoes the permutation).

Do not "improve" the flush scheduling: measured exec time is
deterministic per compiled artifact and bimodal (~343-363 us for
exactly this schedule vs 406-434 us for every variant tried: flush
lag 2-3, ring alternation, end-of-program flush bursts, pair-split
chunks, SWDGE-issued flushes). Even an end-of-program-only change
flipped mid-stream DMA behavior, so the sensitivity is a global
artifact/phase effect, not a local scheduling race; treat this exact
instruction stream as load-bearing and re-benchmark any change.
"""

import os
import sys

import numpy as np

for _p in ("/opt/trn_rl_repo",):
    if os.path.isdir(_p) and _p not in sys.path:
        sys.path.insert(0, _p)

import concourse.bacc as bacc
import concourse.bass as bass
import concourse.tile as tile
from concourse import mybir
from concourse.bass_utils import run_bass_kernel_spmd


def _ensure_ntff_hook():
    """Install the axon NTFF profile hook if the image's antenv lacks it.

    Mirrors trn_boot.py section 6; makes run(..., trace=True) return
    exec_time_ns + perfetto trace instead of silently skipping.
    """
    import types

    try:
        from antenv.axon_hooks import get_axon_ntff_profile_hook  # noqa: F401

        return
    except ImportError:
        pass
    try:
        import antenv
        from trn_agent_boot.trn_boot import _ntff_profile_via_ctypes

        mod = types.ModuleType("antenv.axon_hooks")
        _state = {"hook": None}
        mod.set_axon_ntff_profile_hook = lambda h: _state.__setitem__("hook", h)
        mod.get_axon_ntff_profile_hook = lambda: _state["hook"]
        sys.modules["antenv.axon_hooks"] = mod
        antenv.axon_hooks = mod
        so_path = "/opt/axon/libaxon_pjrt.so"
        if os.path.exists(so_path):
            mod.set_axon_ntff_profile_hook(_ntff_profile_via_ctypes(so_path))
    except Exception:
        pass


_ensure_ntff_hook()

N_VERTICES = 500_000
R = 8
F = 64
N_CORES = 8
N_SHARD = N_VERTICES // N_CORES  # 62500 vertices per core
P = 128  # SBUF partitions
VPP = 8  # vertices per partition per full tile
TILE_V = P * VPP  # 1024 vertices per tile
N_FULL = N_SHARD // TILE_V  # 61 full tiles
TAIL = N_SHARD - N_FULL * TILE_V  # 36 leftover vertices
N_SLOTS = N_FULL * VPP  # 488 staged vertex slots per partition

# Warm-up: small sub-tiles so compute starts as soon as a quarter tile
# has landed instead of idling ~6us behind the first full-tile DMA.
WARMUP = [2, 2, 4]
# Cool-down: the last tiles are split small so the final tile's serial
# DMA->square->reduce->gather->out-DMA chain (the drain) stays short.
COOLDOWN = [4, 4, 2, 2]
# Full tiles after warm-up are processed in pairs that share one
# min/is_le + gather pass, amortizing instruction dispatch overheads.
GROUP = 2


def _block_schedule():
    """[(v0, slot0, width), ...] covering all slots.

    Warm-up/cool-down entries have one sub-tile of width<VPP slots; main
    entries are groups of up to GROUP full tiles (width = n*VPP slots).
    """
    blocks = []
    slot = 0
    for vpp in WARMUP:
        blocks.append((slot * P, slot, vpp))
        slot += vpp
    cool = sum(COOLDOWN)
    while slot < N_SLOTS - cool:
        width = min(GROUP * VPP, N_SLOTS - cool - slot)
        blocks.append((slot * P, slot, width))
        slot += width
    for vpp in COOLDOWN:
        blocks.append((slot * P, slot, vpp))
        slot += vpp
    assert slot == N_SLOTS
    return blocks


BLOCKS = _block_schedule()

_DT = mybir.dt.float32
_AX = mybir.AxisListType
_OP = mybir.AluOpType


def _build_nc():
    nc = bacc.Bacc(
        "TRN2",
        target_bir_lowering=False,
        debug=False,
        enable_asserts=False,
        num_devices=N_CORES,
    )
    x = nc.dram_tensor("inputs", [N_SHARD, R, F], _DT, kind="ExternalInput")
    # Partition-major staged output: raw[p, t*VPP+v, r] = out[t*TILE_V +
    # p*VPP + v, r]; the host undoes the permutation.
    raw = nc.dram_tensor("raw", [P, N_SLOTS, R], _DT, kind="ExternalOutput")
    traw = nc.dram_tensor("traw", [TAIL, R], _DT, kind="ExternalOutput")
    xa = x.ap()

    with tile.TileContext(nc) as tc:
        with (
            tc.tile_pool(name="xin", bufs=7) as xin_pool,
            tc.tile_pool(name="sqd", bufs=3) as sqd_pool,
            tc.tile_pool(name="x8", bufs=3) as x8_pool,
            tc.tile_pool(name="work", bufs=3) as work_pool,
            tc.tile_pool(name="gat", bufs=1) as gat_pool,
            tc.tile_pool(name="stage", bufs=1) as stage_pool,
        ):
            stage = stage_pool.tile([P, N_SLOTS, R], _DT)

            GW = GROUP * VPP  # group width in slots

            def head(xt, pc, vpp, sq_dst, x8_dst):
                """Per-tile: ACT square + f32 gather-column copy (frees
                the big input buffer after two stages), DVE f32 F-reduce
                into the group's norm tile."""
                sqd = sqd_pool.tile([P, VPP, R, F], _DT, tag="sqd")
                nc.scalar.square(sqd[:pc, :vpp], xt[:pc, :vpp])
                nc.scalar.copy(x8_dst, xt[:pc, :vpp, :, 0:R])
                nc.vector.tensor_reduce(
                    out=sq_dst, in_=sqd[:pc, :vpp], axis=_AX.X, op=_OP.add
                )

            def gather(pc, w, sqg, x8g, ot_dst):
                """Per-group min / one-hot (DVE, small) and one-hot
                gather (GpSimd: per-head mult + 3-level add tree)."""
                m = work_pool.tile([P, GW], _DT, tag="m")
                nc.vector.tensor_reduce(
                    out=m[:pc, :w], in_=sqg, axis=_AX.X, op=_OP.min
                )
                # One-hot at the min norm (multi-hot only on bitwise-equal
                # ties, which have ~0 probability for random f32 sums).
                # Stays on DVE: comparison ALU ops on the Pool engine fail
                # to compile (bisected: NEFF backend rejects them).
                sel = work_pool.tile([P, GW, R], _DT, tag="sel")
                nc.vector.tensor_tensor(
                    out=sel[:pc, :w],
                    in0=sqg,
                    in1=m[:pc, :w, None].broadcast_to([pc, w, R]),
                    op=_OP.is_le,
                )
                # Gather via one-hot weighted sum over the first R feature
                # columns (argmin index is always < R), on GpSimd.
                g = gat_pool.tile([P, GW, R, R], _DT, tag="g")
                nc.gpsimd.tensor_tensor(
                    out=g[:pc, :w],
                    in0=x8g,
                    in1=sel[:pc, :w, None, :].broadcast_to([pc, w, R, R]),
                    op=_OP.mult,
                )
                t1 = gat_pool.tile([P, GW, R, 4], _DT, tag="t1")
                nc.gpsimd.tensor_tensor(
                    out=t1[:pc, :w],
                    in0=g[:pc, :w, :, 0:4],
                    in1=g[:pc, :w, :, 4:8],
                    op=_OP.add,
                )
                t2 = gat_pool.tile([P, GW, R, 2], _DT, tag="t2")
                nc.gpsimd.tensor_tensor(
                    out=t2[:pc, :w],
                    in0=t1[:pc, :w, :, 0:2],
                    in1=t1[:pc, :w, :, 2:4],
                    op=_OP.add,
                )
                nc.gpsimd.tensor_tensor(
                    out=ot_dst,
                    in0=t2[:pc, :w, :, 0],
                    in1=t2[:pc, :w, :, 1],
                    op=_OP.add,
                )

            def heads_of(v0, width, pc, tail_vpp):
                if tail_vpp is not None or width < VPP:
                    return [(v0, tail_vpp if tail_vpp is not None else width)]
                return [
                    (v0 + k * TILE_V, VPP) for k in range(width // VPP)
                ]

            def issue_dmas(idx, v0, width, pc, tail_vpp=None):
                """Allocate the block's input tiles and ring the DMA
                doorbells. Runs PREFETCH blocks ahead of the compute so
                the triggers on the scalar ring never queue behind ACT's
                square/copy work for the current block."""
                tiles = []
                for k, (v0k, vpp) in enumerate(heads_of(v0, width, pc, tail_vpp)):
                    xt = xin_pool.tile([P, VPP, R, F], _DT, tag="xt")
                    src = xa[v0k : v0k + pc * vpp].rearrange(
                        "(p v) r f -> p v r f", p=pc
                    )
                    dma_eng = nc.sync if (idx + k) % 2 == 0 else nc.scalar
                    dma_eng.dma_start(out=xt[:pc, :vpp], in_=src)
                    tiles.append(xt)
                return tiles

            def compute_block(tiles, v0, width, pc, ot_dst, tail_vpp=None):
                sqg = work_pool.tile([P, GW, R], _DT, tag="sq")
                x8g = x8_pool.tile([P, GW, R, R], _DT, tag="x8")
                hs = heads_of(v0, width, pc, tail_vpp)
                w = sum(vpp for _, vpp in hs)
                for k, ((_v0k, vpp), xt) in enumerate(zip(hs, tiles)):
                    s0 = k * VPP
                    head(
                        xt, pc, vpp,
                        sqg[:pc, s0 : s0 + vpp],
                        x8g[:pc, s0 : s0 + vpp],
                    )
                gather(pc, w, sqg[:pc, :w], x8g[:pc, :w], ot_dst)

            # Block list with the tail inserted just before the cool-down
            # so its serial chain hides under the cool-down stream.
            work_items = []
            for i, (v0, slot0, width) in enumerate(BLOCKS):
                work_items.append((v0, slot0, width, P, None))
                if TAIL and i == len(BLOCKS) - len(COOLDOWN) - 1:
                    work_items.append(
                        (N_FULL * TILE_V, None, VPP, TAIL, 1)
                    )

            # Chunk boundaries (in slots) for the staged-output DMA. Each
            # chunk is issued one block after its region completes so its
            # semaphore wait is nearly satisfied when the sync sequencer
            # reaches it and barely head-of-line blocks input triggers.
            cool = sum(COOLDOWN)
            fracs = [0.2, 0.4, 0.6, 0.8, 0.94]
            bounds = sorted(
                {round(N_SLOTS * fr / VPP) * VPP for fr in fracs}
                | {N_SLOTS - cool, N_SLOTS}
            )
            chunks = list(zip([0] + bounds[:-1], bounds))
            n_items = len(work_items)
            issue_at = {}
            for a, b in chunks:
                done_i = next(
                    i
                    for i, (_, s0, wd, _pc, tv) in enumerate(work_items)
                    if tv is None and s0 + wd >= b
                )
                issue_at.setdefault(min(done_i + 1, n_items - 1), []).append(
                    (a, b)
                )

            PREFETCH = 2  # blocks of DMA issued ahead of their compute
            ot_tail = None
            pending = []
            dma_idx = 0

            def run_compute(i):
                nonlocal ot_tail
                v0, slot0, width, pc, tv = work_items[i]
                tiles = pending.pop(0)
                if tv is not None:
                    ot_tail = work_pool.tile([P, VPP, R], _DT, tag="ot_tail")
                    compute_block(tiles, v0, width, pc, ot_tail[:pc, :1], tv)
                else:
                    compute_block(
                        tiles, v0, width, pc,
                        stage[:, slot0 : slot0 + width],
                    )
                for a, b in issue_at.get(i, []):
                    nc.sync.dma_start(out=raw.ap()[:, a:b], in_=stage[:, a:b])

            ci = 0
            for i, (v0, slot0, width, pc, tv) in enumerate(work_items):
                pending.append(issue_dmas(dma_idx, v0, width, pc, tv))
                dma_idx += max(1, width // VPP)
                if i >= PREFETCH:
                    run_compute(ci)
                    ci += 1
            while ci < n_items:
                run_compute(ci)
                ci += 1

            if TAIL:
                nc.sync.dma_start(out=traw.ap(), in_=ot_tail[:TAIL, :1])

    nc.finalize()
    return nc


_NC_CACHE = None


def _get_nc():
    global _NC_CACHE
    if _NC_CACHE is None:
        _NC_CACHE = _build_nc()
    return _NC_CACHE


def _decode_raw(raw_arr: np.ndarray, traw_arr: np.ndarray) -> np.ndarray:
    """Map staged [P, N_SLOTS, R] output back to f32 vertex order."""
    raw_f = np.asarray(raw_arr).astype(np.float32)
    parts = []
    for _v0, slot0, width in BLOCKS:
        if width < VPP:
            parts.append(raw_f[:, slot0 : slot0 + width].reshape(P * width, R))
        else:
            for k in range(width // VPP):
                s = slot0 + k * VPP
                parts.append(raw_f[:, s : s + VPP].reshape(P * VPP, R))
    parts.append(np.asarray(traw_arr).astype(np.float32))
    return np.concatenate(parts, axis=0)


def run(inputs: np.ndarray, **spmd_kwargs):
    inputs = np.ascontiguousarray(np.asarray(inputs, dtype=np.float32))
    assert inputs.shape == (N_VERTICES, R, F), inputs.shape
    shards = np.split(inputs, N_CORES, axis=0)
    in_maps = [{"inputs": np.ascontiguousarray(s)} for s in shards]
    res = run_bass_kernel_spmd(
        _get_nc(), in_maps, core_ids=list(range(N_CORES)), **spmd_kwargs
    )
    out = np.concatenate(
        [_decode_raw(r["raw"], r["traw"]) for r in res.results], axis=0
    )
    return out, res


def kernel(inputs: np.ndarray) -> np.ndarray:
    out, _ = run(inputs)
    return out


# revision 26
# speedup vs baseline: 1.0033x; 1.0033x over previous
"""Trainium2 Bass kernel for AngularMinPooling.

out[v, r] = inputs[v, r, argmin_j ||inputs[v, j, :]||_2]
Input (500000, 8, 64) f32 -> Output (500000, 8) f32.
Vertices are sharded across 8 NeuronCores; no cross-core communication.

The problem is memory-bound: 128 MB of input per core against the
per-core HBM share (~358 GB/s nominal, ~373 GB/s delivered here), so
the whole design is about keeping the 16 SDMA queues saturated while
every compute engine runs with slack. Per 128x8-vertex tile: ACT
squares the features into a scratch tile (f32 end-to-end: fp16/bf16
squares flip ~0.02-0.15% of argmins and push rel-err to 1.7e-2..5e-2,
too close to the 2e-2 gate) and copies the first R feature columns to a
small f32 gather tile. The copy exists so the big input buffer frees
after two ACT-stages -- when the gather instead read the input tile
directly, the buffer lived through the whole 6-stage chain, the DMA
runway shrank to ~2 tiles, and the sync sequencer spent ~68 us
head-of-line blocked on buffer-free semaphores (433 us vs 343). DVE
does only single-port work -- the big segmented f32 sum-reduce to
squared norms (4.0 us/tile, the dominant DVE cost; tensor_reduce has
only a 1x uop so 16-bit inputs would not speed it up), the min-reduce,
and the small is_le one-hot -- while the one-hot gather (mult plus a
3-level add tree, measured 2.7 cyc/elem) runs on GpSimd. GpSimd shares
only DVE's *second* SBUF port and tensor_reduce is a 1-port
instruction, so the two engines stream concurrently without contention
(is_le is the lone 2-port DVE op; it must stay on DVE -- comparison
ALU ops on the Pool engine fail to compile). Input DMAs are issued
PREFETCH=2 blocks ahead of their compute so the triggers on the scalar
ring never queue behind ACT's square/copy dispatch (this prefetch took
410 us to 343 us). That leaves DVE ~86%, ACT ~79%, GpSimd ~44% busy
with DMA queues 91% occupied: DMA is the bottleneck and the pipeline
has no stall limit-cycle. Output is staged in f32 SBUF and written out
in chunks on the sync ring one block after each region completes,
partition-major (the host undoes the permutation).

Do not "improve" the flush scheduling: measured exec time was
bimodal across artifacts in interleaved A/Bs (~343-363 us for exactly
this schedule vs 406-434 us for every variant tried: flush lag 2-3,
ring alternation, end-of-program flush bursts, pair-split chunks,
SWDGE-issued flushes); treat this exact instruction stream as
load-bearing and re-benchmark any change. Separately, the HOST
environment drifts: late in a long benching session the same binary
degraded 343 -> 363 -> 391 -> 419 ns while SDMA engine 15 ran ~21%
slower per descriptor than engines 0-14 (379 vs 320 us busy), pacing
the whole pipeline; on a healthy device all 16 engines run even
(~465 ns per 16 KB descriptor) and this kernel measured 342.8/343.8.
A one-DMA-per-group variant (4 MB transfers, 32 KB/partition lines,
half the transfer count) measured dead even with this kernel on the
degraded device (415.7 vs 415.8) but has no healthy-device
measurement, so the proven artifact ships.
"""

import os
import sys

import numpy as np

for _p in ("/opt/trn_rl_repo",):
    if os.path.isdir(_p) and _p not in sys.path:
        sys.path.insert(0, _p)

import concourse.bacc as bacc
import concourse.bass as bass
import concourse.tile as tile
from concourse import mybir
from concourse.bass_utils import run_bass_kernel_spmd


def _ensure_ntff_hook():
    """Install the axon NTFF profile hook if the image's antenv lacks it.

    Mirrors trn_boot.py section 6; makes run(..., trace=True) return
    exec_time_ns + perfetto trace instead of silently skipping.
    """
    import types

    try:
        from antenv.axon_hooks import get_axon_ntff_profile_hook  # noqa: F401

        return
    except ImportError:
        pass
    try:
        import antenv
        from trn_agent_boot.trn_boot import _ntff_profile_via_ctypes

        mod = types.ModuleType("antenv.axon_hooks")
        _state = {"hook": None}
        mod.set_axon_ntff_profile_hook = lambda h: _state.__setitem__("hook", h)
        mod.get_axon_ntff_profile_hook = lambda: _state["hook"]
        sys.modules["antenv.axon_hooks"] = mod
        antenv.axon_hooks = mod
        so_path = "/opt/axon/libaxon_pjrt.so"
        if os.path.exists(so_path):
            mod.set_axon_ntff_profile_hook(_ntff_profile_via_ctypes(so_path))
    except Exception:
        pass


_ensure_ntff_hook()

N_VERTICES = 500_000
R = 8
F = 64
N_CORES = 8
N_SHARD = N_VERTICES // N_CORES  # 62500 vertices per core
P = 128  # SBUF partitions
VPP = 8  # vertices per partition per full tile
TILE_V = P * VPP  # 1024 vertices per tile
N_FULL = N_SHARD // TILE_V  # 61 full tiles
TAIL = N_SHARD - N_FULL * TILE_V  # 36 leftover vertices
N_SLOTS = N_FULL * VPP  # 488 staged vertex slots per partition

# Warm-up: small sub-tiles so compute starts as soon as a quarter tile
# has landed instead of idling ~6us behind the first full-tile DMA.
WARMUP = [2, 2, 4]
# Cool-down: the last tiles are split small so the final tile's serial
# DMA->square->reduce->gather->out-DMA chain (the drain) stays short.
COOLDOWN = [4, 4, 2, 2]
# Full tiles after warm-up are processed in pairs that share one
# min/is_le + gather pass, amortizing instruction dispatch overheads.
GROUP = 2


def _block_schedule():
    """[(v0, slot0, width), ...] covering all slots.

    Warm-up/cool-down entries have one sub-tile of width<VPP slots; main
    entries are groups of up to GROUP full tiles (width = n*VPP slots).
    """
    blocks = []
    slot = 0
    for vpp in WARMUP:
        blocks.append((slot * P, slot, vpp))
        slot += vpp
    cool = sum(COOLDOWN)
    while slot < N_SLOTS - cool:
        width = min(GROUP * VPP, N_SLOTS - cool - slot)
        blocks.append((slot * P, slot, width))
        slot += width
    for vpp in COOLDOWN:
        blocks.append((slot * P, slot, vpp))
        slot += vpp
    assert slot == N_SLOTS
    return blocks


BLOCKS = _block_schedule()

_DT = mybir.dt.float32
_AX = mybir.AxisListType
_OP = mybir.AluOpType


def _build_nc():
    nc = bacc.Bacc(
        "TRN2",
        target_bir_lowering=False,
        debug=False,
        enable_asserts=False,
        num_devices=N_CORES,
    )
    x = nc.dram_tensor("inputs", [N_SHARD, R, F], _DT, kind="ExternalInput")
    # Partition-major staged output: raw[p, t*VPP+v, r] = out[t*TILE_V +
    # p*VPP + v, r]; the host undoes the permutation.
    raw = nc.dram_tensor("raw", [P, N_SLOTS, R], _DT, kind="ExternalOutput")
    traw = nc.dram_tensor("traw", [TAIL, R], _DT, kind="ExternalOutput")
    xa = x.ap()

    with tile.TileContext(nc) as tc:
        with (
            tc.tile_pool(name="xin", bufs=7) as xin_pool,
            tc.tile_pool(name="sqd", bufs=3) as sqd_pool,
            tc.tile_pool(name="x8", bufs=3) as x8_pool,
            tc.tile_pool(name="work", bufs=3) as work_pool,
            tc.tile_pool(name="gat", bufs=1) as gat_pool,
            tc.tile_pool(name="stage", bufs=1) as stage_pool,
        ):
            stage = stage_pool.tile([P, N_SLOTS, R], _DT)

            GW = GROUP * VPP  # group width in slots

            def head(xt, pc, vpp, sq_dst, x8_dst):
                """Per-tile: ACT square + f32 gather-column copy (frees
                the big input buffer after two stages), DVE f32 F-reduce
                into the group's norm tile."""
                sqd = sqd_pool.tile([P, VPP, R, F], _DT, tag="sqd")
                nc.scalar.square(sqd[:pc, :vpp], xt[:pc, :vpp])
                nc.scalar.copy(x8_dst, xt[:pc, :vpp, :, 0:R])
                nc.vector.tensor_reduce(
                    out=sq_dst, in_=sqd[:pc, :vpp], axis=_AX.X, op=_OP.add
                )

            def gather(pc, w, sqg, x8g, ot_dst):
                """Per-group min / one-hot (DVE, small) and one-hot
                gather (GpSimd: per-head mult + 3-level add tree)."""
                m = work_pool.tile([P, GW], _DT, tag="m")
                nc.vector.tensor_reduce(
                    out=m[:pc, :w], in_=sqg, axis=_AX.X, op=_OP.min
                )
                # One-hot at the min norm (multi-hot only on bitwise-equal
                # ties, which have ~0 probability for random f32 sums).
                # Stays on DVE: comparison ALU ops on the Pool engine fail
                # to compile (bisected: NEFF backend rejects them).
                sel = work_pool.tile([P, GW, R], _DT, tag="sel")
                nc.vector.tensor_tensor(
                    out=sel[:pc, :w],
                    in0=sqg,
                    in1=m[:pc, :w, None].broadcast_to([pc, w, R]),
                    op=_OP.is_le,
                )
                # Gather via one-hot weighted sum over the first R feature
                # columns (argmin index is always < R), on GpSimd.
                g = gat_pool.tile([P, GW, R, R], _DT, tag="g")
                nc.gpsimd.tensor_tensor(
                    out=g[:pc, :w],
                    in0=x8g,
                    in1=sel[:pc, :w, None, :].broadcast_to([pc, w, R, R]),
                    op=_OP.mult,
                )
                t1 = gat_pool.tile([P, GW, R, 4], _DT, tag="t1")
                nc.gpsimd.tensor_tensor(
                    out=t1[:pc, :w],
                    in0=g[:pc, :w, :, 0:4],
                    in1=g[:pc, :w, :, 4:8],
                    op=_OP.add,
                )
                t2 = gat_pool.tile([P, GW, R, 2], _DT, tag="t2")
                nc.gpsimd.tensor_tensor(
                    out=t2[:pc, :w],
                    in0=t1[:pc, :w, :, 0:2],
                    in1=t1[:pc, :w, :, 2:4],
                    op=_OP.add,
                )
                nc.gpsimd.tensor_tensor(
                    out=ot_dst,
                    in0=t2[:pc, :w, :, 0],
                    in1=t2[:pc, :w, :, 1],
                    op=_OP.add,
                )

            def heads_of(v0, width, pc, tail_vpp):
                if tail_vpp is not None or width < VPP:
                    return [(v0, tail_vpp if tail_vpp is not None else width)]
                return [
                    (v0 + k * TILE_V, VPP) for k in range(width // VPP)
                ]

            def issue_dmas(idx, v0, width, pc, tail_vpp=None):
                """Allocate the block's input tiles and ring the DMA
                doorbells. Runs PREFETCH blocks ahead of the compute so
                the triggers on the scalar ring never queue behind ACT's
                square/copy work for the current block."""
                tiles = []
                for k, (v0k, vpp) in enumerate(heads_of(v0, width, pc, tail_vpp)):
                    xt = xin_pool.tile([P, VPP, R, F], _DT, tag="xt")
                    src = xa[v0k : v0k + pc * vpp].rearrange(
                        "(p v) r f -> p v r f", p=pc
                    )
                    dma_eng = nc.sync if (idx + k) % 2 == 0 else nc.scalar
                    dma_eng.dma_start(out=xt[:pc, :vpp], in_=src)
                    tiles.append(xt)
                return tiles

            def compute_block(tiles, v0, width, pc, ot_dst, tail_vpp=None):
                sqg = work_pool.tile([P, GW, R], _DT, tag="sq")
                x8g = x8_pool.tile([P, GW, R, R], _DT, tag="x8")
                hs = heads_of(v0, width, pc, tail_vpp)
                w = sum(vpp for _, vpp in hs)
                for k, ((_v0k, vpp), xt) in enumerate(zip(hs, tiles)):
                    s0 = k * VPP
                    head(
                        xt, pc, vpp,
                        sqg[:pc, s0 : s0 + vpp],
                        x8g[:pc, s0 : s0 + vpp],
                    )
                gather(pc, w, sqg[:pc, :w], x8g[:pc, :w], ot_dst)

            # Block list with the tail inserted just before the cool-down
            # so its serial chain hides under the cool-down stream.
            work_items = []
            for i, (v0, slot0, width) in enumerate(BLOCKS):
                work_items.append((v0, slot0, width, P, None))
                if TAIL and i == len(BLOCKS) - len(COOLDOWN) - 1:
                    work_items.append(
                        (N_FULL * TILE_V, None, VPP, TAIL, 1)
                    )

            # Chunk boundaries (in slots) for the staged-output DMA. Each
            # chunk is issued one block after its region completes so its
            # semaphore wait is nearly satisfied when the sync sequencer
            # reaches it and barely head-of-line blocks input triggers.
            cool = sum(COOLDOWN)
            fracs = [0.2, 0.4, 0.6, 0.8, 0.94]
            bounds = sorted(
                {round(N_SLOTS * fr / VPP) * VPP for fr in fracs}
                | {N_SLOTS - cool, N_SLOTS}
            )
            chunks = list(zip([0] + bounds[:-1], bounds))
            n_items = len(work_items)
            issue_at = {}
            for a, b in chunks:
                done_i = next(
                    i
                    for i, (_, s0, wd, _pc, tv) in enumerate(work_items)
                    if tv is None and s0 + wd >= b
                )
                issue_at.setdefault(min(done_i + 1, n_items - 1), []).append(
                    (a, b)
                )

            PREFETCH = 2  # blocks of DMA issued ahead of their compute
            ot_tail = None
            pending = []
            dma_idx = 0

            def run_compute(i):
                nonlocal ot_tail
                v0, slot0, width, pc, tv = work_items[i]
                tiles = pending.pop(0)
                if tv is not None:
                    ot_tail = work_pool.tile([P, VPP, R], _DT, tag="ot_tail")
                    compute_block(tiles, v0, width, pc, ot_tail[:pc, :1], tv)
                else:
                    compute_block(
                        tiles, v0, width, pc,
                        stage[:, slot0 : slot0 + width],
                    )
                for a, b in issue_at.get(i, []):
                    nc.sync.dma_start(out=raw.ap()[:, a:b], in_=stage[:, a:b])

            ci = 0
            for i, (v0, slot0, width, pc, tv) in enumerate(work_items):
                pending.append(issue_dmas(dma_idx, v0, width, pc, tv))
                dma_idx += max(1, width // VPP)
                if i >= PREFETCH:
                    run_compute(ci)
                    ci += 1
            while ci < n_items:
                run_compute(ci)
                ci += 1

            if TAIL:
                nc.sync.dma_start(out=traw.ap(), in_=ot_tail[:TAIL, :1])

    nc.finalize()
    return nc


_NC_CACHE = None


def _get_nc():
    global _NC_CACHE
    if _NC_CACHE is None:
        _NC_CACHE = _build_nc()
    return _NC_CACHE


def _decode_raw(raw_arr: np.ndarray, traw_arr: np.ndarray) -> np.ndarray:
    """Map staged [P, N_SLOTS, R] output back to f32 vertex order."""
    raw_f = np.asarray(raw_arr).astype(np.float32)
    parts = []
    for _v0, slot0, width in BLOCKS:
        if width < VPP:
            parts.append(raw_f[:, slot0 : slot0 + width].reshape(P * width, R))
        else:
            for k in range(width // VPP):
                s = slot0 + k * VPP
                parts.append(raw_f[:, s : s + VPP].reshape(P * VPP, R))
    parts.append(np.asarray(traw_arr).astype(np.float32))
    return np.concatenate(parts, axis=0)


def run(inputs: np.ndarray, **spmd_kwargs):
    inputs = np.ascontiguousarray(np.asarray(inputs, dtype=np.float32))
    assert inputs.shape == (N_VERTICES, R, F), inputs.shape
    shards = np.split(inputs, N_CORES, axis=0)
    in_maps = [{"inputs": np.ascontiguousarray(s)} for s in shards]
    res = run_bass_kernel_spmd(
        _get_nc(), in_maps, core_ids=list(range(N_CORES)), **spmd_kwargs
    )
    out = np.concatenate(
        [_decode_raw(r["raw"], r["traw"]) for r in res.results], axis=0
    )
    return out, res


def kernel(inputs: np.ndarray) -> np.ndarray:
    out, _ = run(inputs)
    return out


# revision 27
# speedup vs baseline: 1.1291x; 1.1254x over previous
"""Trainium2 Bass kernel for AngularMinPooling.

out[v, r] = inputs[v, r, argmin_j ||inputs[v, j, :]||_2]
Input (500000, 8, 64) f32 -> Output (500000, 8) f32.
Vertices are sharded across 8 NeuronCores; no cross-core communication.

The problem is memory-bound: 128 MB of input per core against the
per-core HBM share (~358 GB/s nominal, ~373 GB/s delivered here), so
the whole design is about keeping the 16 SDMA queues saturated while
every compute engine runs with slack. Per 128x8-vertex tile: ACT
squares the features into a scratch tile (f32 end-to-end: fp16/bf16
squares flip ~0.02-0.15% of argmins and push rel-err to 1.7e-2..5e-2,
too close to the 2e-2 gate) and copies the first R feature columns to a
small f32 gather tile. The copy exists so the big input buffer frees
after two ACT-stages -- when the gather instead read the input tile
directly, the buffer lived through the whole 6-stage chain, the DMA
runway shrank to ~2 tiles, and the sync sequencer spent ~68 us
head-of-line blocked on buffer-free semaphores (433 us vs 343). DVE
does only single-port work -- the big segmented f32 sum-reduce to
squared norms (4.0 us/tile, the dominant DVE cost; tensor_reduce has
only a 1x uop so 16-bit inputs would not speed it up), the min-reduce,
and the small is_le one-hot -- while the one-hot gather (mult plus a
3-level add tree, measured 2.7 cyc/elem) runs on GpSimd. GpSimd shares
only DVE's *second* SBUF port and tensor_reduce is a 1-port
instruction, so the two engines stream concurrently without contention
(is_le is the lone 2-port DVE op; it must stay on DVE -- comparison
ALU ops on the Pool engine fail to compile). Input DMAs are issued
PREFETCH=2 blocks ahead of their compute so the triggers on the scalar
ring never queue behind ACT's square/copy dispatch (this prefetch took
410 us to 343 us). That leaves DVE ~86%, ACT ~79%, GpSimd ~44% busy
with DMA queues 91% occupied: DMA is the bottleneck and the pipeline
has no stall limit-cycle. Output is staged in f32 SBUF and written out
in chunks on the sync ring one block after each region completes,
partition-major (the host undoes the permutation).

Do not "improve" the flush scheduling: measured exec time was
bimodal across artifacts in interleaved A/Bs (~343-363 us for exactly
this schedule vs 406-434 us for every variant tried: flush lag 2-3,
ring alternation, end-of-program flush bursts, pair-split chunks,
SWDGE-issued flushes); treat this exact instruction stream as
load-bearing and re-benchmark any change. Separately, the HOST
environment drifts: late in a long benching session the same binary
degraded 343 -> 363 -> 391 -> 419 ns while SDMA engine 15 ran ~21%
slower per descriptor than engines 0-14 (379 vs 320 us busy), pacing
the whole pipeline; on a healthy device all 16 engines run even
(~465 ns per 16 KB descriptor) and this kernel measured 342.8/343.8.
A one-DMA-per-group variant (4 MB transfers, 32 KB/partition lines,
half the transfer count) measured dead even with this kernel on the
degraded device (415.7 vs 415.8) but has no healthy-device
measurement, so the proven artifact ships.
"""

import os
import sys

import numpy as np

for _p in ("/opt/trn_rl_repo",):
    if os.path.isdir(_p) and _p not in sys.path:
        sys.path.insert(0, _p)

import concourse.bacc as bacc
import concourse.bass as bass
import concourse.tile as tile
from concourse import mybir
from concourse.bass_utils import run_bass_kernel_spmd


def _ensure_ntff_hook():
    """Install the axon NTFF profile hook if the image's antenv lacks it.

    Mirrors trn_boot.py section 6; makes run(..., trace=True) return
    exec_time_ns + perfetto trace instead of silently skipping.
    """
    import types

    try:
        from antenv.axon_hooks import get_axon_ntff_profile_hook  # noqa: F401

        return
    except ImportError:
        pass
    try:
        import antenv
        from trn_agent_boot.trn_boot import _ntff_profile_via_ctypes

        mod = types.ModuleType("antenv.axon_hooks")
        _state = {"hook": None}
        mod.set_axon_ntff_profile_hook = lambda h: _state.__setitem__("hook", h)
        mod.get_axon_ntff_profile_hook = lambda: _state["hook"]
        sys.modules["antenv.axon_hooks"] = mod
        antenv.axon_hooks = mod
        so_path = "/opt/axon/libaxon_pjrt.so"
        if os.path.exists(so_path):
            mod.set_axon_ntff_profile_hook(_ntff_profile_via_ctypes(so_path))
    except Exception:
        pass


_ensure_ntff_hook()

N_VERTICES = 500_000
R = 8
F = 64
N_CORES = 8
N_SHARD = N_VERTICES // N_CORES  # 62500 vertices per core
P = 128  # SBUF partitions
VPP = 8  # vertices per partition per full tile
TILE_V = P * VPP  # 1024 vertices per tile
N_FULL = N_SHARD // TILE_V  # 61 full tiles
TAIL = N_SHARD - N_FULL * TILE_V  # 36 leftover vertices
N_SLOTS = N_FULL * VPP  # 488 staged vertex slots per partition

# Warm-up: small sub-tiles so compute starts as soon as a quarter tile
# has landed instead of idling ~6us behind the first full-tile DMA.
WARMUP = [2, 2, 4]
# Cool-down: the last tiles are split small so the final tile's serial
# DMA->square->reduce->gather->out-DMA chain (the drain) stays short.
COOLDOWN = [4, 4, 2, 2]
# Full tiles after warm-up are processed in pairs that share one
# min/is_le + gather pass, amortizing instruction dispatch overheads.
GROUP = 2


def _block_schedule():
    """[(v0, slot0, width), ...] covering all slots.

    Warm-up/cool-down entries have one sub-tile of width<VPP slots; main
    entries are groups of up to GROUP full tiles (width = n*VPP slots).
    """
    blocks = []
    slot = 0
    for vpp in WARMUP:
        blocks.append((slot * P, slot, vpp))
        slot += vpp
    cool = sum(COOLDOWN)
    while slot < N_SLOTS - cool:
        width = min(GROUP * VPP, N_SLOTS - cool - slot)
        blocks.append((slot * P, slot, width))
        slot += width
    for vpp in COOLDOWN:
        blocks.append((slot * P, slot, vpp))
        slot += vpp
    assert slot == N_SLOTS
    return blocks


BLOCKS = _block_schedule()

_DT = mybir.dt.float32
_AX = mybir.AxisListType
_OP = mybir.AluOpType


def _build_nc():
    nc = bacc.Bacc(
        "TRN2",
        target_bir_lowering=False,
        debug=False,
        enable_asserts=False,
        num_devices=N_CORES,
    )
    x = nc.dram_tensor("inputs", [N_SHARD, R, F], _DT, kind="ExternalInput")
    # Partition-major staged output: raw[p, t*VPP+v, r] = out[t*TILE_V +
    # p*VPP + v, r]; the host undoes the permutation.
    raw = nc.dram_tensor("raw", [P, N_SLOTS, R], _DT, kind="ExternalOutput")
    traw = nc.dram_tensor("traw", [TAIL, R], _DT, kind="ExternalOutput")
    xa = x.ap()

    with tile.TileContext(nc) as tc:
        with (
            tc.tile_pool(name="xin", bufs=7) as xin_pool,
            tc.tile_pool(name="sqd", bufs=3) as sqd_pool,
            tc.tile_pool(name="x8", bufs=3) as x8_pool,
            tc.tile_pool(name="work", bufs=3) as work_pool,
            tc.tile_pool(name="gat", bufs=1) as gat_pool,
            tc.tile_pool(name="stage", bufs=1) as stage_pool,
        ):
            stage = stage_pool.tile([P, N_SLOTS, R], _DT)

            GW = GROUP * VPP  # group width in slots

            def head(xt, pc, vpp, sq_dst, x8_dst):
                """Per-tile: ACT square + f32 gather-column copy (frees
                the big input buffer after two stages), DVE f32 F-reduce
                into the group's norm tile."""
                sqd = sqd_pool.tile([P, VPP, R, F], _DT, tag="sqd")
                nc.scalar.square(sqd[:pc, :vpp], xt[:pc, :vpp])
                nc.scalar.copy(x8_dst, xt[:pc, :vpp, :, 0:R])
                nc.vector.tensor_reduce(
                    out=sq_dst, in_=sqd[:pc, :vpp], axis=_AX.X, op=_OP.add
                )

            def gather(pc, w, sqg, x8g, ot_dst):
                """Per-group min / one-hot (DVE, small) and one-hot
                gather (GpSimd: per-head mult + 3-level add tree)."""
                m = work_pool.tile([P, GW], _DT, tag="m")
                nc.vector.tensor_reduce(
                    out=m[:pc, :w], in_=sqg, axis=_AX.X, op=_OP.min
                )
                # One-hot at the min norm (multi-hot only on bitwise-equal
                # ties, which have ~0 probability for random f32 sums).
                # Stays on DVE: comparison ALU ops on the Pool engine fail
                # to compile (bisected: NEFF backend rejects them).
                sel = work_pool.tile([P, GW, R], _DT, tag="sel")
                nc.vector.tensor_tensor(
                    out=sel[:pc, :w],
                    in0=sqg,
                    in1=m[:pc, :w, None].broadcast_to([pc, w, R]),
                    op=_OP.is_le,
                )
                # Gather via one-hot weighted sum over the first R feature
                # columns (argmin index is always < R), on GpSimd.
                g = gat_pool.tile([P, GW, R, R], _DT, tag="g")
                nc.gpsimd.tensor_tensor(
                    out=g[:pc, :w],
                    in0=x8g,
                    in1=sel[:pc, :w, None, :].broadcast_to([pc, w, R, R]),
                    op=_OP.mult,
                )
                t1 = gat_pool.tile([P, GW, R, 4], _DT, tag="t1")
                nc.gpsimd.tensor_tensor(
                    out=t1[:pc, :w],
                    in0=g[:pc, :w, :, 0:4],
                    in1=g[:pc, :w, :, 4:8],
                    op=_OP.add,
                )
                t2 = gat_pool.tile([P, GW, R, 2], _DT, tag="t2")
                nc.gpsimd.tensor_tensor(
                    out=t2[:pc, :w],
                    in0=t1[:pc, :w, :, 0:2],
                    in1=t1[:pc, :w, :, 2:4],
                    op=_OP.add,
                )
                nc.gpsimd.tensor_tensor(
                    out=ot_dst,
                    in0=t2[:pc, :w, :, 0],
                    in1=t2[:pc, :w, :, 1],
                    op=_OP.add,
                )

            def heads_of(v0, width, pc, tail_vpp):
                if tail_vpp is not None or width < VPP:
                    return [(v0, tail_vpp if tail_vpp is not None else width)]
                return [
                    (v0 + k * TILE_V, VPP) for k in range(width // VPP)
                ]

            def issue_dmas(idx, v0, width, pc, tail_vpp=None):
                """Allocate the block's input tiles and ring the DMA
                doorbells. Runs PREFETCH blocks ahead of the compute so
                the triggers on the scalar ring never queue behind ACT's
                square/copy work for the current block."""
                tiles = []
                for k, (v0k, vpp) in enumerate(heads_of(v0, width, pc, tail_vpp)):
                    xt = xin_pool.tile([P, VPP, R, F], _DT, tag="xt")
                    src = xa[v0k : v0k + pc * vpp].rearrange(
                        "(p v) r f -> p v r f", p=pc
                    )
                    dma_eng = nc.sync if (idx + k) % 2 == 0 else nc.scalar
                    dma_eng.dma_start(out=xt[:pc, :vpp], in_=src)
                    tiles.append(xt)
                return tiles

            def compute_block(tiles, v0, width, pc, ot_dst, tail_vpp=None):
                sqg = work_pool.tile([P, GW, R], _DT, tag="sq")
                x8g = x8_pool.tile([P, GW, R, R], _DT, tag="x8")
                hs = heads_of(v0, width, pc, tail_vpp)
                w = sum(vpp for _, vpp in hs)
                for k, ((_v0k, vpp), xt) in enumerate(zip(hs, tiles)):
                    s0 = k * VPP
                    head(
                        xt, pc, vpp,
                        sqg[:pc, s0 : s0 + vpp],
                        x8g[:pc, s0 : s0 + vpp],
                    )
                gather(pc, w, sqg[:pc, :w], x8g[:pc, :w], ot_dst)

            # Block list with the tail inserted just before the cool-down
            # so its serial chain hides under the cool-down stream.
            work_items = []
            for i, (v0, slot0, width) in enumerate(BLOCKS):
                work_items.append((v0, slot0, width, P, None))
                if TAIL and i == len(BLOCKS) - len(COOLDOWN) - 1:
                    work_items.append(
                        (N_FULL * TILE_V, None, VPP, TAIL, 1)
                    )

            # Chunk boundaries (in slots) for the staged-output DMA. Each
            # chunk is issued one block after its region completes so its
            # semaphore wait is nearly satisfied when the sync sequencer
            # reaches it and barely head-of-line blocks input triggers.
            cool = sum(COOLDOWN)
            fracs = [0.2, 0.4, 0.6, 0.8, 0.94]
            bounds = sorted(
                {round(N_SLOTS * fr / VPP) * VPP for fr in fracs}
                | {N_SLOTS - cool, N_SLOTS}
            )
            chunks = list(zip([0] + bounds[:-1], bounds))
            n_items = len(work_items)
            issue_at = {}
            for a, b in chunks:
                done_i = next(
                    i
                    for i, (_, s0, wd, _pc, tv) in enumerate(work_items)
                    if tv is None and s0 + wd >= b
                )
                # Chunks past the 0.94 mark issue at the LAST compute
                # step: issued at done_i+1 they sit on the sync ring
                # ahead of the final cool-down input triggers with a
                # not-yet-satisfied gather wait, head-of-line blocking
                # them (8.5 us idle on the critical DMA engine).
                step = (
                    min(done_i + 1, n_items - 1)
                    if b <= N_SLOTS - cool
                    else n_items - 1
                )
                issue_at.setdefault(step, []).append((a, b))

            PREFETCH = 2  # blocks of DMA issued ahead of their compute
            ot_tail = None
            pending = []
            dma_idx = 0

            def run_compute(i):
                nonlocal ot_tail
                v0, slot0, width, pc, tv = work_items[i]
                tiles = pending.pop(0)
                if tv is not None:
                    ot_tail = work_pool.tile([P, VPP, R], _DT, tag="ot_tail")
                    compute_block(tiles, v0, width, pc, ot_tail[:pc, :1], tv)
                else:
                    compute_block(
                        tiles, v0, width, pc,
                        stage[:, slot0 : slot0 + width],
                    )
                for a, b in issue_at.get(i, []):
                    nc.sync.dma_start(out=raw.ap()[:, a:b], in_=stage[:, a:b])

            ci = 0
            for i, (v0, slot0, width, pc, tv) in enumerate(work_items):
                pending.append(issue_dmas(dma_idx, v0, width, pc, tv))
                dma_idx += max(1, width // VPP)
                if i >= PREFETCH:
                    run_compute(ci)
                    ci += 1
            while ci < n_items:
                run_compute(ci)
                ci += 1

            if TAIL:
                nc.sync.dma_start(out=traw.ap(), in_=ot_tail[:TAIL, :1])

    nc.finalize()
    return nc


_NC_CACHE = None


def _get_nc():
    global _NC_CACHE
    if _NC_CACHE is None:
        _NC_CACHE = _build_nc()
    return _NC_CACHE


def _decode_raw(raw_arr: np.ndarray, traw_arr: np.ndarray) -> np.ndarray:
    """Map staged [P, N_SLOTS, R] output back to f32 vertex order."""
    raw_f = np.asarray(raw_arr).astype(np.float32)
    parts = []
    for _v0, slot0, width in BLOCKS:
        if width < VPP:
            parts.append(raw_f[:, slot0 : slot0 + width].reshape(P * width, R))
        else:
            for k in range(width // VPP):
                s = slot0 + k * VPP
                parts.append(raw_f[:, s : s + VPP].reshape(P * VPP, R))
    parts.append(np.asarray(traw_arr).astype(np.float32))
    return np.concatenate(parts, axis=0)


def run(inputs: np.ndarray, **spmd_kwargs):
    inputs = np.ascontiguousarray(np.asarray(inputs, dtype=np.float32))
    assert inputs.shape == (N_VERTICES, R, F), inputs.shape
    shards = np.split(inputs, N_CORES, axis=0)
    in_maps = [{"inputs": np.ascontiguousarray(s)} for s in shards]
    res = run_bass_kernel_spmd(
        _get_nc(), in_maps, core_ids=list(range(N_CORES)), **spmd_kwargs
    )
    out = np.concatenate(
        [_decode_raw(r["raw"], r["traw"]) for r in res.results], axis=0
    )
    return out, res


def kernel(inputs: np.ndarray) -> np.ndarray:
    out, _ = run(inputs)
    return out
